# revision 29
# baseline (speedup 1.0000x reference)
"""BandSplit (gather -> per-band MLP -> scatter-add OLA -> /ola) on 8 TRN2 cores.

Strategy
--------
The whole reference computation is linear in x (the per-band pre/post weights,
melbank weights, mask, scatter-add and the final /ola are all linear maps, and
the biases contribute an x-independent constant).  On the host we fold all of
it into a single matrix A of shape (C*F, C*F) mapping the (c, f) spectrum of
one (b, t) token to the (c, f) output spectrum:

    out[b, :, t, :] = A^T @ vec(x[b, :, t, :]) + const

Because every mel band covers a *contiguous* frequency range of width <= Wmax,
A is block-banded: A[(ci, fi), (co, fo)] == 0 unless |fi - fo| < Wmax.  The
device kernel is therefore a banded matmul, data-parallel over the 4096
(b, t) tokens across the 8 NeuronCores (512 tokens/core) with zero
cross-core communication.

The kernel is HBM-bound on the input side (x bf16 + packed band matrix
~3.9 MB/core at ~250-315 GB/s effective), so the matmul stream is ordered
ki-MAJOR across all four token chunks: consumption order exactly matches
DMA arrival order (ab slab, then its x blocks, ...) and the PE computes
while most of the input is still in flight.  PSUM fits because bank
lifetimes in ki are disjoint enough to share: per chunk, banks (0,2,4)
share one PSUM slot and (1,3) another -- exactly 8 banks across 4 chunks.
The host pre-casts x to bf16 and pre-transposes it into the exact SBUF
layout the matmuls need (f on partitions, tokens on columns); the 9th
f-chunk (f >= 1024) has only 2 live rows and ships as a tiny side tensor
into a zeroed tile.  An ~8 us warmup burst of junk matmuls bridges
engine-boot to first-data and guarantees the free-running HAM clock-gate
window latches the PE at 2.4 GHz before the stream begins (shorter bursts
frequently fail to latch and the stream runs at 1.2 GHz).  Loads issue
from the scalar (ACT) HWDGE ring, which boots ~1 us before sync; early
stores ride the sync ring and the final per-chunk stores ride ACT so
nothing queues behind anything.  Per-bank drains (f32 PSUM -> f16 SBUF)
alternate between the vector/scalar engines as each bank's last ki
completes; the ki7+ki8 tail runs chunk-major so each chunk's final drain
and store overlap the later chunks' matmuls.
"""

import numpy as np

_P = 128


def _fold_matrix(pre_w, pre_b, post_w, post_b, idx, melw, mask, ola_window):
    """Fold the full reference computation into (A, const).

    A: (C, F, C, F) with out[co, fo] = sum_{ci, fi} x[ci, fi] * A[ci, fi, co, fo]
    const: (C, F) additive constant from the biases.
    """
    K, W = idx.shape
    C = 2
    F = ola_window.shape[0]

    pre_w = np.asarray(pre_w, np.float64)
    post_w = np.asarray(post_w, np.float64)
    pre_b = np.asarray(pre_b, np.float64)
    post_b = np.asarray(post_b, np.float64)
    wts = (np.asarray(melw, np.float64) * np.asarray(mask, np.float64))
    msk = np.asarray(mask, np.float64)
    idx = np.asarray(idx)

    # Per-band folded linear map: M[k, i=(w,cin), j=(w',cout)]
    M = np.einsum('kio,koj->kij', pre_w, post_w).reshape(K, W, C, W, C)
    vals = M * wts[:, :, None, None, None] * msk[:, None, None, :, None]

    fin = idx[:, :, None, None, None].astype(np.int64)
    fout = idx[:, None, None, :, None].astype(np.int64)
    cin = np.arange(C)[None, None, :, None, None]
    cout = np.arange(C)[None, None, None, None, :]
    flat = ((cin * F + fin) * C + cout) * F + fout
    A = np.bincount(
        np.broadcast_to(flat, vals.shape).ravel(), weights=vals.ravel(),
        minlength=C * F * C * F,
    ).reshape(C, F, C, F)
    A /= ola_window[None, None, None, :]

    # Bias constant: (sum_o pre_b[k,o] * post_w[k,o,(w',co)] + post_b) * mask, /ola
    bv = (np.einsum('ko,koj->kj', pre_b, post_w) + post_b).reshape(K, W, C)
    bv = bv * msk[:, :, None]
    cflat = (np.arange(C)[None, None, :] * F + idx[:, :, None]).astype(np.int64)
    const = np.bincount(
        np.broadcast_to(cflat, bv.shape).ravel(), weights=bv.ravel(),
        minlength=C * F,
    ).reshape(C, F)
    const /= ola_window[None, :]
    return A, const


_PROGRAM_CACHE = {}

_F_OUT = 1025
_C = 2
_KI = 9
_KX = 8                    # ki chunks shipped in xs (ki=8 rides the side tensor)
_F_PAD = _KI * _P          # 1152
_TCH = 4                   # token chunks (of 128) per core
_PS_W = _C * _F_OUT        # 2050 PSUM output columns (co interleaved: fo*C+co)
_BANKS = [(b * 512, min(_PS_W, (b + 1) * 512)) for b in range((_PS_W + 511) // 512)]
_KCOL = _TCH * _C * _P     # 1024 x-cols per ki block (tch, ci, tok)
_TCORE = _TCH * _P         # 512 tokens per core

# PSUM slot sharing: bank lifetimes in ki are (0: ki0-2, 1: ki1-4, 2: ki3-6,
# 3: ki5-8, 4: ki7-8), so (0,2,4) share one slot per chunk and (1,3) another.
_SLOT_TAG = {0: "A", 1: "B", 2: "A", 3: "B", 4: "A"}


def _build_program(offs, TW, wins, n_cores):
    """Build the Bass/Tile program. Returns the compiled Bass object."""
    import concourse.bass as bass
    import concourse.tile as tile
    import concourse.mybir as mybir
    from concourse import bacc

    f32 = mybir.dt.float32
    bf16 = mybir.dt.bfloat16
    f16 = mybir.dt.float16
    P = _P
    KI = _KI
    C = _C
    XCOLS = _KX * _KCOL           # 8192 cols (ki-major, both pairs adjacent)
    X8C = C * _TCORE              # 1024 cols of the ki=8 side tile

    nc = bacc.Bacc("TRN2", target_bir_lowering=False, debug=False,
                   num_devices=n_cores)
    # xs: pre-transposed bf16, col = ki*1024 + tch*256 + ci*128 + tok (ki<8)
    xs = nc.dram_tensor("xs", [P, XCOLS], bf16, kind="ExternalInput")
    # x8: the two live rows of f-chunk 8 (f=1024 bin, bias ones); col =
    # ci*512 + tch*128 + tok
    x8 = nc.dram_tensor("x8", [2, X8C], bf16, kind="ExternalInput")
    # ab: packed band windows [P, TW] (ki, ci at offsets offs; co interleaved)
    ab = nc.dram_tensor("ab", [P, TW], bf16, kind="ExternalInput")
    # y: channel-interleaved f16 (col = fo*C + co), chunk-minor; host
    # de-interleaves
    y = nc.dram_tensor("y", [P, _TCH, _PS_W], f16, kind="ExternalOutput")

    def segments(ki):
        lo, hi = 2 * wins[ki][0], 2 * wins[ki][1]
        segs = []
        for b, (bs, be) in enumerate(_BANKS):
            s, e = max(lo, bs), min(hi, be)
            if s < e:
                segs.append((b, s, e))
        return segs

    # per-bank (ki, ci, s, e) touch order within one chunk's MM stream, and
    # the last ki touching each bank (drain point)
    touches = {}
    last_ki = {}
    for ki in range(KI):
        for ci in range(C):
            for (b, s, e) in segments(ki):
                touches.setdefault(b, []).append((ki, ci, s, e))
                last_ki[b] = ki

    with tile.TileContext(nc) as tc:
        with (
            tc.tile_pool(name="apool", bufs=1) as apool,
            tc.tile_pool(name="xpool", bufs=1) as xpool,
            tc.tile_pool(name="opool", bufs=1) as opool,
            tc.tile_pool(name="idpool", bufs=1) as idpool,
            tc.tile_pool(name="pspool", bufs=1, space="PSUM") as pspool,
        ):
            abig = apool.tile([P, TW], bf16, name="abig")
            xbig = xpool.tile([P, XCOLS], bf16, name="xbig")
            t8 = idpool.tile([P, X8C], bf16, name="t8")
            junk = idpool.tile([P, P], bf16, name="junk")

            # the last-needed x blocks (ki 6,7) ride SWDGE from GpSimd,
            # which boots ~2.4us before the ACT ring: they move during the
            # otherwise-idle HBM window and shrink the ACT load stream
            nc.gpsimd.dma_start(xbig[:, 6 * _KCOL:7 * _KCOL],
                                xs[:, 6 * _KCOL:7 * _KCOL])
            nc.gpsimd.dma_start(xbig[:, 7 * _KCOL:8 * _KCOL],
                                xs[:, 7 * _KCOL:8 * _KCOL])
            nc.gpsimd.memset(t8[:], 0.0)
            nc.vector.memset(junk[:], 0.0)

            # loads on the ACT (scalar) HWDGE ring, interleaved in the order
            # the ki-major stream consumes: ab slab (3 ki), then that
            # triple's x blocks.  ab slabs stay coarse for DMA efficiency.
            # The SP ring is kept for stores so they never queue behind loads.
            for g in range(3):
                o0 = offs[(3 * g, 0)]
                o1 = offs[(3 * g + 3, 0)] if g < 2 else TW
                nc.scalar.dma_start(abig[:, o0:o1], ab[:, o0:o1])
                for ki in range(3 * g, min(3 * g + 3, 6)):
                    nc.scalar.dma_start(
                        xbig[:, ki * _KCOL:(ki + 1) * _KCOL],
                        xs[:, ki * _KCOL:(ki + 1) * _KCOL])
            nc.scalar.dma_start(t8[0:2, :], x8[:, :])

            # warmup burst: the HAM clock-gate window is free-running, so the
            # burst must cover a full ~3.4us window to latch 2.4 GHz before
            # the stream begins; it also bridges engine-boot -> first data.
            # The warm target shares the "A0" PSUM slot as instance #0.
            warm = pspool.tile([P, 512], f32, tag="A0", name="warm")
            for _ in range(96):
                nc.tensor.matmul(warm[:, :P], junk[:], junk[:],
                                 start=True, stop=True)

            def a_tile(ci, ki):
                o = offs[(ki, ci)]
                return abig[:, o:o + 2 * (wins[ki][1] - wins[ki][0])]

            # per-chunk state: PSUM tile per live bank; one output
            # staging tile holds all four chunks side by side
            cur = {}                   # (tch, b) -> PSUM tile
            ot = opool.tile([P, _TCH, _PS_W], f16, name="ot")
            drain_rr = [0]             # round-robin DVE/ACT for drains

            def drain(tch, b):
                bs, be = _BANKS[b]
                t = cur.pop((tch, b))
                if drain_rr[0] % 2 == 0:
                    nc.vector.tensor_copy(ot[:, tch, bs:be], t[:])
                else:
                    nc.scalar.copy(ot[:, tch, bs:be], t[:])
                drain_rr[0] += 1

            def emit_mms(ki, tch):
                lo2 = 2 * wins[ki][0]
                for ci in range(C):
                    if ki < _KX:
                        o = ki * _KCOL + tch * C * P + ci * P
                        lhsT = xbig[:, o:o + P]
                    else:
                        o = ci * _TCORE + tch * P
                        lhsT = t8[:, o:o + P]
                    for (b, s, e) in segments(ki):
                        if (tch, b) not in cur:
                            bs, be = _BANKS[b]
                            cur[(tch, b)] = pspool.tile(
                                [P, be - bs], f32,
                                tag=f"{_SLOT_TAG[b]}{tch}",
                                name=f"bk{b}_{tch}")
                        order = touches[b]
                        first = order[0] == (ki, ci, s, e)
                        last = order[-1] == (ki, ci, s, e)
                        bs = _BANKS[b][0]
                        nc.tensor.matmul(
                            cur[(tch, b)][:, s - bs:e - bs],
                            lhsT,
                            a_tile(ci, ki)[:, s - lo2:e - lo2],
                            start=first, stop=last,
                        )

            # ki-major over ki 0..6: consumption order == DMA arrival order
            for ki in range(7):
                for tch in range(_TCH):
                    emit_mms(ki, tch)
                # drain every bank whose last ki just completed, and ship
                # each output piece once its banks are in SBUF
                for b in range(len(_BANKS)):
                    if last_ki[b] == ki:
                        for tch in range(_TCH):
                            drain(tch, b)
                if ki == 4:            # banks 0,1 drained -> [0:1024)
                    for tch in range(_TCH):
                        nc.sync.dma_start(y[:, tch, :1024], ot[:, tch, :1024])
                elif ki == 6:          # bank 2 drained -> [1024:1536)
                    for tch in range(_TCH):
                        nc.sync.dma_start(y[:, tch, 1024:1536],
                                          ot[:, tch, 1024:1536])

            # tail (ki 7-8) chunk-major: each chunk's bank-3/4 drains and
            # final store overlap the later chunks' matmuls
            for tch in range(_TCH):
                emit_mms(7, tch)
                emit_mms(8, tch)
                drain(tch, 3)
                drain(tch, 4)
                nc.scalar.dma_start(y[:, tch, 1536:], ot[:, tch, 1536:])

    nc.compile()
    return nc


def kernel(**inputs):
    import ml_dtypes

    x = np.ascontiguousarray(np.asarray(inputs["x"], np.float32))
    B, C, T, F = x.shape
    assert (B, C, F) == (4, 2, 1025), (B, C, F)
    N_CORES = 8
    TS = T // N_CORES                        # 128 frames per core

    A, const = _fold_matrix(
        inputs["pre_w"], inputs["pre_b"], inputs["post_w"], inputs["post_b"],
        inputs["idx"], inputs["melw"], inputs["mask"], inputs["ola_window"],
    )

    # padded A, with the bias constant folded into spare row F (ci = 0)
    Apad = np.zeros((C, _F_PAD, C, _F_PAD), np.float32)
    Apad[:, :F, :, :F] = A.astype(np.float32)
    Apad[0, F, :, :F] = const.astype(np.float32)

    # exact nonzero column window per 128-row chunk (same for all channel blocks)
    nz = (Apad != 0).any(axis=(0, 2))          # (F_PAD rows, F_PAD cols)
    wins = []
    for ki in range(_KI):
        cols = nz[ki * _P:(ki + 1) * _P].any(axis=0)
        nzc = np.nonzero(cols)[0]
        if len(nzc) == 0:
            lo, hi = ki * _P, ki * _P + 1
        else:
            lo, hi = int(nzc[0]), int(nzc[-1]) + 1
        wins.append((lo, min(hi, F)))
    # coverage: every output column [0, F) must be written by >= 1 matmul
    covered = np.zeros(_F_PAD, bool)
    for lo, hi in wins:
        covered[lo:hi] = True
    assert covered[:F].all(), "window coverage hole"

    # packed band layout: offsets per (ki, ci); the two output channels are
    # interleaved along columns (col = fo * C + co) to match the PSUM layout
    offs = {}
    tw = 0
    for ki in range(_KI):
        w2 = (2 * (wins[ki][1] - wins[ki][0]) + 15) // 16 * 16
        for ci in range(C):
            offs[(ki, ci)] = tw
            tw += w2
    TW = tw

    ab = np.zeros((_P, TW), ml_dtypes.bfloat16)
    for ki in range(_KI):
        lo, hi = wins[ki]
        for ci in range(C):
            o = offs[(ki, ci)]
            blk = Apad[ci, ki * _P:(ki + 1) * _P, :, lo:hi]       # (P, C, w)
            ab[:, o:o + 2 * (hi - lo)] = blk.transpose(0, 2, 1).reshape(_P, -1)

    key = (TW, tuple(wins), N_CORES)
    if key not in _PROGRAM_CACHE:
        _PROGRAM_CACHE[key] = _build_program(offs, TW, wins, N_CORES)
    nc = _PROGRAM_CACHE[key]

    # host-side bf16 cast + transpose into device layout (ki < 8):
    # xs[p, ki*1024 + tch*256 + ci*128 + t] = xq[tch, ci, mTS+t, 128ki+p]
    xq = x.astype(ml_dtypes.bfloat16)

    in_maps = []
    for m in range(N_CORES):
        sl = xq[:, :, m * TS:(m + 1) * TS, :1024]     # (B=tch, C, 128, 1024)
        sl = sl.reshape(_TCH, C, TS, _KX, _P)         # (tch, ci, t, ki, p)
        xs_m = np.ascontiguousarray(
            sl.transpose(4, 3, 0, 1, 2)               # (p, ki, tch, ci, t)
        ).reshape(_P, _KX * _KCOL)
        # side tensor: row 0 = f=1024 bin, row 1 = bias ones; col = ci*512+tok
        x8_m = np.empty((2, C * _TCORE), ml_dtypes.bfloat16)
        x8_m[0] = np.ascontiguousarray(
            xq[:, :, m * TS:(m + 1) * TS, 1024].transpose(1, 0, 2)
        ).reshape(C * _TCORE)
        x8_m[1] = np.float32(1.0)
        in_maps.append({"xs": xs_m, "x8": x8_m, "ab": ab})

    # bass_utils imports antenv.axon_hooks when tracing is requested; this
    # image lacks that module, so provide a no-op stub if it's missing.
    try:
        import antenv.axon_hooks  # noqa: F401
    except ImportError:
        import sys
        import types
        import antenv
        stub = types.ModuleType("antenv.axon_hooks")
        stub.get_axon_ntff_profile_hook = lambda: None
        stub.set_axon_ntff_profile_hook = lambda h: None
        sys.modules["antenv.axon_hooks"] = stub
        antenv.axon_hooks = stub

    from concourse.bass_utils import run_bass_kernel_spmd
    res = run_bass_kernel_spmd(nc, in_maps, core_ids=list(range(N_CORES)))
    globals()["_LAST_RESULT"] = res

    out = np.empty((B, C, T, F), np.float32)
    for m in range(N_CORES):
        # y: (P, TCH, F*C) interleaved; chunk tch == batch tch of this t-slice
        ym = res.results[m]["y"].astype(np.float32).reshape(_P, _TCH, F, C)
        ym = ym.transpose(1, 3, 0, 2)                 # (b, c, t, f)
        out[:, :, m * TS:(m + 1) * TS, :] = ym
    return out
